# revision 25
# baseline (speedup 1.0000x reference)
"""BoxConv2d Trainium2 kernel.

Math: the reference (integral image + bilinear interpolation of fractional
box corners) is algebraically identical to, for each (c, f):

    out[b, c*F+f] = A_cf @ X[b, c] @ B_cf^T

with closed-form interpolation-x-cumsum matrices

    A_cf[h, i] = clip(u1(c,f,h) - i, 0, 1) - clip(u0(c,f,h) - i, 0, 1)
    B_cf[w', j] = clip(v1(c,f,w') - j, 0, 1) - clip(v0(c,f,w') - j, 0, 1)

where u0 = clip(h + x_min*H, 0, H), u1 = clip(h + x_max*H + 1, 0, H) etc.
The tiny A/B matrices are built on host from the box parameters; all
per-sample compute runs on device as dense matmuls on the PE.

Both stages run in bf16 hi/lo split form (x = xh + xl captures ~16 mantissa
bits; products AhXh + AlXh + AhXl accumulate in fp32 PSUM), which runs the
PE at full speed instead of fp32's quarter speed. Measured output error vs
the fp32 reference is ~5e-6 of the output scale.

Stage order is col-interp first (stationary = X^T, shared over all 8
filters), then row-interp (stationary = A^T, shared over all 8 batch
samples) — this keeps every matmul's moving operand at N=512 and minimizes
stationary reloads.

Sharding: channel-parallel — core k handles c in [4k, 4k+4) for all b, f.
Each core reads its x slice + A/B slices (~6 MiB) and writes a contiguous
16 MiB output-channel block.
"""

import numpy as np

import concourse.bacc as bacc
import concourse.mybir as mybir
import concourse.tile as tile
from concourse import bass_utils

B, C, F, H, W = 8, 32, 8, 128, 128
NCORES = 8
CPC = C // NCORES  # channels per core = 4
FP = mybir.dt.float32
BF = mybir.dt.bfloat16

_cache = {}


def _build_program():
    if "nc" in _cache:
        return _cache["nc"]

    nc = bacc.Bacc("TRN2", target_bir_lowering=False, debug=False)

    # x^T per (b,c) as [j, b, i] (hi/lo bf16), col matrices B^T as
    # [j, f, w'], row matrices A^T as [i, f, h].
    xth_d = nc.dram_tensor("xth", [CPC, W, B * H], BF, kind="ExternalInput").ap()
    xtl_d = nc.dram_tensor("xtl", [CPC, W, B * H], BF, kind="ExternalInput").ap()
    bth_d = nc.dram_tensor("bth", [CPC, W, F * W], BF, kind="ExternalInput").ap()
    btl_d = nc.dram_tensor("btl", [CPC, W, F * W], BF, kind="ExternalInput").ap()
    ath_d = nc.dram_tensor("ath", [CPC, H, F * H], BF, kind="ExternalInput").ap()
    atl_d = nc.dram_tensor("atl", [CPC, H, F * H], BF, kind="ExternalInput").ap()
    out_d = nc.dram_tensor("out", [B, CPC * F, H, W], FP, kind="ExternalOutput").ap()

    with tile.TileContext(nc) as tc:
        with (
            tc.tile_pool(name="wp", bufs=4) as wp,
            tc.tile_pool(name="zp", bufs=4) as zp,
            tc.tile_pool(name="op", bufs=4) as op,
            tc.tile_pool(name="pzp", bufs=4, space="PSUM") as pzp,
            tc.tile_pool(name="pop", bufs=4, space="PSUM") as pop,
        ):
            state = {}

            def emit_s1_load(c):
                xth_t = wp.tile([W, B * H], BF, tag="xth", name=f"xth_{c}")
                bth_t = wp.tile([W, F * W], BF, tag="bth", name=f"bth_{c}")
                xtl_t = wp.tile([W, B * H], BF, tag="xtl", name=f"xtl_{c}")
                btl_t = wp.tile([W, F * W], BF, tag="btl", name=f"btl_{c}")
                if c == 0:
                    # fine-grained: land exactly what the first matmul group
                    # needs first, so the PE starts ~4us earlier
                    nc.sync.dma_start(xth_t[:, 0:H], xth_d[c][:, 0:H])
                    nc.sync.dma_start(bth_t[:, 0:512], bth_d[c][:, 0:512])
                    nc.sync.dma_start(btl_t[:, 0:512], btl_d[c][:, 0:512])
                    nc.sync.dma_start(xtl_t[:, 0:H], xtl_d[c][:, 0:H])
                    nc.sync.dma_start(bth_t[:, 512:], bth_d[c][:, 512:])
                    nc.sync.dma_start(btl_t[:, 512:], btl_d[c][:, 512:])
                    nc.sync.dma_start(xth_t[:, H:], xth_d[c][:, H:])
                    nc.sync.dma_start(xtl_t[:, H:], xtl_d[c][:, H:])
                else:
                    nc.sync.dma_start(xth_t, xth_d[c])
                    nc.sync.dma_start(bth_t, bth_d[c])
                    nc.sync.dma_start(xtl_t, xtl_d[c])
                    nc.sync.dma_start(btl_t, btl_d[c])
                # Z_c[i, (b, f, w')] = sum_j X[b,c][i, j] * B[c,f][w', j],
                # kept as a bf16 hi/lo pair for stage 2.
                zh_t = zp.tile([H, B * F * W], BF, tag="zh", name=f"zh_{c}")
                zl_t = zp.tile([H, B * F * W], BF, tag="zl", name=f"zl_{c}")
                state[c] = [xth_t, xtl_t, bth_t, btl_t, None, None, zh_t, zl_t]

            def emit_s2_load(c):
                # A^T matrices aren't needed until stage 2 — keep them off
                # the startup critical path.
                ath_t = wp.tile([H, F * H], BF, tag="ath", name=f"ath_{c}")
                nc.sync.dma_start(ath_t, ath_d[c])
                atl_t = wp.tile([H, F * H], BF, tag="atl", name=f"atl_{c}")
                nc.sync.dma_start(atl_t, atl_d[c])
                state[c][4] = ath_t
                state[c][5] = atl_t

            def emit_s1_group(c, b):
                xth_t, xtl_t, bth_t, btl_t, _, _, zh_t, zl_t = state[c]
                bs = slice(b * H, (b + 1) * H)
                for n0 in (0, 512):
                    ns = slice(n0, n0 + 512)
                    # one PSUM bank per half-group -> finer recycling
                    pz = pzp.tile([H, F * W // 2], FP, tag="pz",
                                  name=f"pz_{c}_{b}_{n0}")
                    nc.tensor.matmul(pz, xth_t[:, bs], bth_t[:, ns],
                                     start=True, stop=False)
                    nc.tensor.matmul(pz, xth_t[:, bs], btl_t[:, ns],
                                     start=False, stop=False)
                    nc.tensor.matmul(pz, xtl_t[:, bs], bth_t[:, ns],
                                     start=False, stop=True)
                    zs = slice(b * F * W + n0, b * F * W + n0 + 512)
                    nc.scalar.copy(zh_t[:, zs], pz)               # ACT: cast hi
                    nc.vector.tensor_sub(zl_t[:, zs], pz, zh_t[:, zs])  # DVE: lo

            def emit_s2_group(c, f, tail=False):
                _, _, _, _, ath_t, atl_t, zh_t, zl_t = state[c]
                zh_v = zh_t.rearrange("i (b f w) -> i b f w", b=B, f=F)
                zl_v = zl_t.rearrange("i (b f w) -> i b f w", b=B, f=F)
                fs = slice(f * H, (f + 1) * H)
                o_t = op.tile([H, B * W], FP, tag="o", name=f"o_{c}_{f}")
                out_v = out_d[:, c * F + f].rearrange("b h w -> h b w")
                o_v = o_t.rearrange("h (b w) -> h b w", b=B)
                for bi in range(0, B, 4):
                    ns = slice(bi * W, (bi + 4) * W)
                    zh_f = zh_v[:, bi : bi + 4, f]
                    zl_f = zl_v[:, bi : bi + 4, f]
                    # one PSUM bank per half-group -> finer recycling
                    po = pop.tile([H, B * W // 2], FP, tag="po",
                                  name=f"po_{c}_{f}_{bi}")
                    nc.tensor.matmul(po, ath_t[:, fs], zh_f,
                                     start=True, stop=False)
                    nc.tensor.matmul(po, atl_t[:, fs], zh_f,
                                     start=False, stop=False)
                    nc.tensor.matmul(po, ath_t[:, fs], zl_f,
                                     start=False, stop=True)
                    eng = nc.vector.tensor_copy if bi == 0 else nc.scalar.copy
                    eng(o_t[:, ns], po)
                    if tail:
                        nc.sync.dma_start(out_v[:, bi : bi + 4], o_v[:, bi : bi + 4])
                if not tail:
                    nc.sync.dma_start(out_v, o_v)

            # Software pipeline: s1 of channel c runs interleaved with s2 of
            # channel c-1 so the PE always has an alternative matmul group
            # while PSUM drains.
            # all weight loads are issued up front: the DMA engines are
            # otherwise idle until the first stores (~18us in), and loads
            # issued mid-kernel steal bandwidth from the store stream,
            # which otherwise backlogs ~13us past the last matmul.
            for c in range(CPC):
                emit_s1_load(c)
                emit_s2_load(c)
            for g in range(B):
                emit_s1_group(0, g)
            for c in range(1, CPC):
                for g in range(B):
                    emit_s1_group(c, g)
                    emit_s2_group(c - 1, g)
            for g in range(B):
                emit_s2_group(CPC - 1, g, tail=(g == B - 1))

    nc.compile()
    _cache["nc"] = nc
    return nc


def _host_mats(x_min, x_max, y_min, y_max, max_h, max_w):
    dt = np.float32
    xm = np.asarray(x_min, dt) * dt(max_h)
    xM = np.asarray(x_max, dt) * dt(max_h)
    ym = np.asarray(y_min, dt) * dt(max_w)
    yM = np.asarray(y_max, dt) * dt(max_w)
    h = np.arange(H, dtype=dt)
    w = np.arange(W, dtype=dt)
    u0 = np.clip(h[None, None, :] + xm[:, :, None], 0.0, dt(max_h))
    u1 = np.clip(h[None, None, :] + xM[:, :, None] + dt(1.0), 0.0, dt(max_h))
    v0 = np.clip(w[None, None, :] + ym[:, :, None], 0.0, dt(max_w))
    v1 = np.clip(w[None, None, :] + yM[:, :, None] + dt(1.0), 0.0, dt(max_w))
    i = np.arange(H, dtype=dt)
    A = np.clip(u1[..., None] - i, 0.0, 1.0) - np.clip(u0[..., None] - i, 0.0, 1.0)
    j = np.arange(W, dtype=dt)
    Bm = np.clip(v1[..., None] - j, 0.0, 1.0) - np.clip(v0[..., None] - j, 0.0, 1.0)
    # At[c, i, f, h] = A[c, f, h, i];  Bt[c, j, f, w'] = B[c, f, w', j]
    At = np.ascontiguousarray(np.transpose(A, (0, 3, 1, 2)), dtype=dt)
    Bt = np.ascontiguousarray(np.transpose(Bm, (0, 3, 1, 2)), dtype=dt)
    return At.reshape(C, H, F * H), Bt.reshape(C, W, F * W)


def _split_bf16(x):
    import ml_dtypes
    hi = x.astype(ml_dtypes.bfloat16)
    lo = (x - hi.astype(np.float32)).astype(ml_dtypes.bfloat16)
    return hi, lo


def _in_maps(input, x_min, x_max, y_min, y_max, max_input_h, max_input_w):
    x = np.asarray(input, np.float32)
    At, Bt = _host_mats(x_min, x_max, y_min, y_max, int(max_input_h),
                        int(max_input_w))
    # xt[c, j, b, i] = x[b, c, i, j]
    xt = np.ascontiguousarray(np.transpose(x, (1, 3, 0, 2))).reshape(C, W, B * H)
    xth, xtl = _split_bf16(xt)
    ath, atl = _split_bf16(At)
    bth, btl = _split_bf16(Bt)
    maps = []
    for k in range(NCORES):
        cs = slice(k * CPC, (k + 1) * CPC)
        maps.append({
            "xth": np.ascontiguousarray(xth[cs]),
            "xtl": np.ascontiguousarray(xtl[cs]),
            "ath": np.ascontiguousarray(ath[cs]),
            "atl": np.ascontiguousarray(atl[cs]),
            "bth": np.ascontiguousarray(bth[cs]),
            "btl": np.ascontiguousarray(btl[cs]),
        })
    return maps


def run(inputs, **spmd_kwargs):
    """Build (cached), run on 8 cores, return (full_out, BassKernelResults)."""
    nc = _build_program()
    maps = _in_maps(**inputs)
    res = bass_utils.run_bass_kernel_spmd(
        nc, maps, core_ids=list(range(NCORES)), **spmd_kwargs
    )
    out = np.empty((B, C * F, H, W), np.float32)
    for k in range(NCORES):
        out[:, k * CPC * F : (k + 1) * CPC * F] = res.results[k]["out"]
    return out, res


def kernel(**inputs) -> np.ndarray:
    out, _ = run(inputs)
    return out
